# revision 45
# baseline (speedup 1.0000x reference)
"""Trainium2 Bass kernel for nn_CombinedLoss (chamfer x2 + MSE).

final = mse(pc1_3, pc2) + 0.5*chamfer(pc1_0, pc2) + chamfer(pc1_1, pc2)

Strategy (8 NeuronCores, SPMD):
  Four KNN "directions" (query set -> target set):
    D0: q=pc2    (16384) t=pc1_0  (16384)   [cd dist1]
    D1: q=pc1_0  (16384) t=pc2    (16384)   [cd dist2]
    D2: q=pc2    (16384) t=pc1_1  (4096)    [seed dist1]
    D3: q=pc1_1  (4096)  t=pc2    (16384)   [seed dist2]
  Queries of each direction are sharded across the 8 cores (row-block of the
  pairwise-distance matrix); each core computes sum(sqrt(min d2)) over its
  query rows entirely on-device, plus a partial MSE sum.  Host just sums the
  8x per-direction scalars and divides (the "all-reduce of means").

  d2 is produced directly by the tensor engine: points are augmented to
  K=13 bf16 hi/lo vectors such that aT@b = |a|^2 + |b|^2 - 2 a.b (exact to
  ~2^-16) and accumulated in fp32 PSUM.  Row-mins are maintained by DVE
  tensor_tensor_reduce ops with running-min chaining; optionally ScalarE
  casts PSUM tiles to fp16 in SBUF so the DVE reduce runs in 2x mode over
  pairs of tiles (4 entries/cycle/lane).
"""

import numpy as np
import ml_dtypes
from contextlib import ExitStack

import bass_rust
import concourse.bass as bass
import concourse.tile as tile
from concourse import mybir
from concourse.bass_utils import run_bass_kernel_spmd
from concourse.vector_clock import ScopedClock


class SplitDrainTileContext(tile.TileContext):
    """TileContext that emits spare bare drains before the tail drain.  The
    tail drain needs ~12 sync waits but HW instructions carry only one
    through this walrus backend; legalize_waits() redistributes the excess
    onto the recorded bare drains (safe: nothing depends on a bare drain)."""

    N_SPARE_DRAINS = 24

    def _drain_and_barrier(self, tick_clock, wait_clock):
        spares = []
        for _ in range(self.N_SPARE_DRAINS):
            d = self.nc.sync.drain()
            spares.append(d.ins.name if hasattr(d, "ins") else d.name)
        self.nc._spare_drain_names = set(spares)
        return super()._drain_and_barrier(tick_clock, wait_clock)

F32 = mybir.dt.float32
F16 = mybir.dt.float16
BF16 = mybir.dt.bfloat16
OP_MIN = mybir.AluOpType.min
OP_ADD = mybir.AluOpType.add
OP_SUB = mybir.AluOpType.subtract
OP_MUL = mybir.AluOpType.mult
AXIS_X = mybir.AxisListType.X
SQRT = mybir.ActivationFunctionType.Sqrt

NCORES = 8
K = 13          # augmented contraction dim
MMN = 512       # matmul free dim (one PSUM bank of fp32)
GRP = 2048      # targets per reduce group (4 banks)
QT = 128        # queries per tile (PE partition dim)
BIGF = 3.0e38

BF = ml_dtypes.bfloat16

# Full-problem config.  Per-core query counts; targets are full.
FULL_CFG = dict(
    nq_pc=2048,      # per-core slice of a 16384-point query set
    nq_11=512,       # per-core slice of the 4096-point query set
    nt_pc=16384,     # full target set size (pc1_0 / pc2)
    nt_11=4096,      # full target set size (pc1_1)
    mse_free=48,     # per-core MSE elements = 128 * mse_free
    # groups per query-tile routed through the ScalarE fp16-cast path
    # (DVE 4x reduce); the rest reduce directly from PSUM on DVE at 1x.
    cast_16k=5,      # for directions with 16384 targets (8 groups/tile)
    cast_4k=1,       # for directions with 4096 targets (2 groups/tile)
)


def build_bass(cfg, debug_taps=False):
    nc = bass.Bass()

    # Tile's tail sem-clear lowers to EVENT_SEMAPHORE_RANGE_CLEAR, which this
    # neuronxcc walrus rejects ("ISA wrong length").  NRT's per-execution
    # preamble already zeroes user semaphores (runtime sema_reset), so skip
    # emitting the clear instructions but keep the allocator bookkeeping.
    def _clear_and_free(sems, _nc=nc):
        if not sems:
            return
        sem_nums = [s.num if hasattr(s, "num") else s for s in sems]
        _nc._state.prepend_free_semaphores(sem_nums)
        for poison_set in _nc._tile_sem_poison_stack:
            poison_set.update(sem_nums)
    nc.clear_and_free_semaphores = _clear_and_free

    nq_pc, nq_11 = cfg["nq_pc"], cfg["nq_11"]
    nt_pc, nt_11 = cfg["nt_pc"], cfg["nt_11"]
    mse_free = cfg["mse_free"]

    d_q2 = nc.declare_dram_parameter("q_pc2", [K, nq_pc], BF16, isOutput=False)
    d_q10 = nc.declare_dram_parameter("q_pc10", [K, nq_pc], BF16, isOutput=False)
    d_q11 = nc.declare_dram_parameter("q_pc11", [K, nq_11], BF16, isOutput=False)
    d_t10 = nc.declare_dram_parameter("t_pc10", [K, nt_pc], BF16, isOutput=False)
    d_t2 = nc.declare_dram_parameter("t_pc2", [K, nt_pc], BF16, isOutput=False)
    d_t11 = nc.declare_dram_parameter("t_pc11", [K, nt_11], BF16, isOutput=False)
    d_ma = nc.declare_dram_parameter("mse_a", [128, mse_free], F32, isOutput=False)
    d_mb = nc.declare_dram_parameter("mse_b", [128, mse_free], F32, isOutput=False)
    d_out = nc.declare_dram_parameter("partials", [1, 8], F32, isOutput=True)
    d_dbg = {}

    # direction table: (query dram, n_queries, target dram, n_targets, cast_groups)
    cast16, cast4 = cfg["cast_16k"], cfg["cast_4k"]
    dirs = [
        (d_q2, nq_pc, d_t10, nt_pc, cast16),
        (d_q10, nq_pc, d_t2, nt_pc, cast16),
        (d_q2, nq_pc, d_t11, nt_11, cast4),
        (d_q11, nq_11, d_t2, nt_pc, cast16),
    ]
    n_tiles = [nq // QT for (_, nq, _, _, _) in dirs]
    n_grps = [nt // GRP for (_, _, _, nt, _) in dirs]
    ntot_tiles = sum(n_tiles)
    # one raw column per (query tile, target group) + one for MSE
    raw_base = []
    acc = 0
    for ntl, ng in zip(n_tiles, n_grps):
        raw_base.append(acc)
        acc += ntl * ng
    n_raw = acc + 1
    mse_col = n_raw - 1

    with SplitDrainTileContext(nc) as tc, ExitStack() as ctx:
        pin = ctx.enter_context(tc.tile_pool(name="pin", bufs=1))
        ppsum = ctx.enter_context(tc.tile_pool(name="ppsum", bufs=2, space="PSUM"))
        pcast = ctx.enter_context(tc.tile_pool(name="pcast", bufs=4))
        prm = ctx.enter_context(tc.tile_pool(name="prm", bufs=4))
        pout = ctx.enter_context(tc.tile_pool(name="pout", bufs=2))

        # --- resident inputs / constants ---
        sb_q = {}
        for name, dram, shape in (
            ("q2", d_q2, [K, nq_pc]),
            ("q10", d_q10, [K, nq_pc]),
            ("q11", d_q11, [K, nq_11]),
            ("t10", d_t10, [K, nt_pc]),
            ("t2", d_t2, [K, nt_pc]),
            ("t11", d_t11, [K, nt_11]),
        ):
            t = pin.tile(shape, BF16, tag=name)
            nc.gpsimd.dma_start(t[:], dram[:])
            sb_q[name] = t
        dram_to_sb = {id(d_q2): "q2", id(d_q10): "q10", id(d_q11): "q11",
                      id(d_t10): "t10", id(d_t2): "t2", id(d_t11): "t11"}

        ma = pin.tile([128, mse_free], F32, tag="ma")
        nc.gpsimd.dma_start(ma[:], d_ma[:])
        mb = pin.tile([128, mse_free], F32, tag="mb")
        nc.gpsimd.dma_start(mb[:], d_mb[:])

        ones = pin.tile([128, 1], F32, tag="ones")
        nc.vector.memset(ones[:], 1.0)
        res_raw = pin.tile([128, n_raw], F32, tag="resraw")
        mins = pin.tile([128, ntot_tiles], F32, tag="mins")
        sums = pin.tile([128, 8], F32, tag="sums")
        nc.vector.memset(sums[:], 0.0)

        # --- DMA-sem observers: each engine observes every input DMA once,
        # so no later compute instruction needs more than one sync wait. ---
        obs = pin.tile([1, 2], F32, tag="obs")
        for oi, t in enumerate((ma, mb)):
            nc.vector.tensor_copy(obs[:, oi:oi + 1], t[0:1, 0:1])
        for name in ("q2", "q10", "q11", "t10", "t2", "t11"):
            t = sb_q[name]
            wps = ppsum.tile([1, 1], F32, tag="grp")
            nc.tensor.matmul(wps[:], lhsT=t[:, 0:1], rhs=t[:, 0:1],
                             start=True, stop=True)

        # --- MSE partial: sum((a-b)^2) per partition -> res_raw[:, mse_col] ---
        diff = pin.tile([128, mse_free], F32, tag="diff")
        nc.vector.tensor_tensor(diff[:], ma[:], mb[:], OP_SUB)
        sq = pin.tile([128, mse_free], F32, tag="sq")
        nc.vector.tensor_tensor(sq[:], diff[:], diff[:], OP_MUL)
        nc.vector.tensor_reduce(res_raw[:, mse_col:mse_col + 1], sq[:],
                                mybir.AxisListType.X, OP_ADD)

        # --- chamfer directions ---
        for di, (d_qd, nq, d_td, nt, ncast) in enumerate(dirs):
            q_sb = sb_q[dram_to_sb[id(d_qd)]]
            t_sb = sb_q[dram_to_sb[id(d_td)]]
            ngrp = n_grps[di]
            ncast_d = min(ncast, ngrp)
            for ti in range(nq // QT):
                q_ap = q_sb[:, ti * QT:(ti + 1) * QT]
                for g in range(ngrp):
                    ps = ppsum.tile([128, GRP], F32, tag="grp")
                    for m in range(GRP // MMN):
                        off = g * GRP + m * MMN
                        nc.tensor.matmul(
                            ps[:, m * MMN:(m + 1) * MMN],
                            lhsT=q_ap, rhs=t_sb[:, off:off + MMN],
                            start=True, stop=True,
                        )
                    gc = raw_base[di] + ti * ngrp + g
                    acc_ap = res_raw[:, gc:gc + 1]
                    if g < ncast_d:
                        ct = pcast.tile([128, GRP], F16, tag="ct")
                        # 1-element ACT toucher: absorbs the WAR-on-slot wait
                        # (vs the DVE reader of the slot's previous tenant) so
                        # the real cast carries only its PE wait (HW instrs
                        # hold a single sync-wait slot).
                        nc.scalar.mul(ct[0:1, 0:1], ct[0:1, 0:1], 0.0)
                        nc.scalar.copy(ct[:], ps[:])
                        to = pout.tile([128, GRP], F16, tag="ttr_out")
                        nc.vector.tensor_scalar(
                            to[:], ct[:], BIGF, None, OP_MIN, OP_MIN,
                            accum_out=acc_ap)
                    else:
                        to = pout.tile([128, GRP], F32, tag="ttr_out")
                        nc.vector.tensor_scalar(
                            to[:], ps[:], BIGF, None, OP_MIN, OP_MIN,
                            accum_out=acc_ap)

        def tap(nm, tl, shape, dt_):
            if debug_taps:
                d = nc.declare_dram_parameter(nm, shape, dt_, isOutput=True)
                nc.sync.dma_start(d[:], tl[:])

        # --- finals: clamp, per-tile min over groups, sqrt, sums ---
        ngc = n_raw - 1
        nc.vector.tensor_scalar_max(res_raw[:, 0:ngc], res_raw[:, 0:ngc], 0.0)
        tap("dbg_raw", res_raw, [128, n_raw], F32)
        c0 = 0
        for di, (ntl, ng) in enumerate(zip(n_tiles, n_grps)):
            base = raw_base[di]
            src = res_raw[:, base:base + ntl * ng]
            src3 = src.rearrange("p (t g) -> p t g", g=ng)
            nc.vector.tensor_reduce(mins[:, c0:c0 + ntl], src3,
                                    mybir.AxisListType.X, OP_MIN)
            c0 += ntl
        nc.scalar.activation(mins[:, 0:ntot_tiles], mins[:, 0:ntot_tiles], SQRT)
        tap("dbg_mins", mins, [128, ntot_tiles], F32)
        c0 = 0
        for d, ntl in enumerate(n_tiles):
            nc.vector.reduce_sum(sums[:, d:d + 1], mins[:, c0:c0 + ntl], axis=AXIS_X)
            c0 += ntl
        nc.vector.tensor_copy(sums[:, 4:5], res_raw[:, mse_col:mse_col + 1])

        ps_fin = ppsum.tile([1, 8], F32, tag="grp")
        nc.tensor.matmul(ps_fin[:], lhsT=ones[:], rhs=sums[:], start=True, stop=True)
        out_sb = pin.tile([1, 8], F32, tag="outsb")
        nc.vector.tensor_copy(out_sb[:], ps_fin[:])
        nc.sync.dma_start(d_out[:], out_sb[:])

    legalize_waits(nc, lenient=debug_taps)
    return nc


WAIT_CAPS = {}
DEFAULT_WAIT_CAP = 1


def legalize_waits(nc, skip_types=("InstDrain",), lenient=False):
    """Cap per-instruction sync waits for the neuronxcc walrus backend.

    HW instruction structs carry a single (wait, update) EVENTS slot; walrus
    rejects instructions (at least matmuls) with more than one wait.  Excess
    waits are hoisted onto an earlier instruction of the same engine that has
    a free wait slot.  Safety: a hoisted wait may only move to a position
    after the instruction whose sem update satisfies it (positions taken in
    global block order = Tile's scheduled order, a valid topological order),
    so the schedule itself remains feasible and no deadlock is introduced.
    """
    f = nc.m.functions[0]
    glob = []
    for blk in f.blocks:
        for inst in blk.instructions:
            glob.append(inst)

    # cumulative sem updates in scheduled order
    from collections import defaultdict
    cum = defaultdict(int)
    hist = defaultdict(list)  # sem id -> [(pos, cum_after)]
    sem_updaters = defaultdict(set)  # sem id -> {(engine, is_dma)}
    for pos, inst in enumerate(glob):
        si = inst.sync_info
        if si is not None and si.on_update:
            is_dma = type(inst).__name__ == "InstDMACopy"
            for u in si.on_update:
                cum[u.id] += u.update_value if u.update_value is not None else 1
                hist[u.id].append((pos, cum[u.id]))
                sem_updaters[u.id].add((inst.engine, is_dma))

    def producer_pos(w):
        for pos, c in hist[w.id]:
            if c >= w.wait_value:
                return pos
        return -1  # satisfied externally / never: be conservative below

    eng_pos = defaultdict(list)  # engine -> [global positions]
    for pos, inst in enumerate(glob):
        eng_pos[inst.engine].append(pos)

    n_waits = {}
    for pos, inst in enumerate(glob):
        si = inst.sync_info
        n_waits[pos] = len(si.on_wait) if si is not None and si.on_wait else 0

    # The tail drain aggregates the whole global clock (~12 waits).  Move its
    # excess waits onto the spare bare drains emitted just before it; nothing
    # depends on a bare drain, so this cannot deadlock.
    spare_names = getattr(nc, "_spare_drain_names", set())
    spares = [i for i in glob if i.name in spare_names]
    si_idx = 0
    for pos, inst in enumerate(glob):
        if type(inst).__name__ != "InstDrain" or inst.name in spare_names:
            continue
        si = inst.sync_info
        if si is None or not si.on_wait or len(si.on_wait) <= 1:
            continue
        waits = list(si.on_wait)
        keep = waits[:1]
        for w in waits[1:]:
            if si_idx >= len(spares):
                keep.append(w)
                continue
            sp = spares[si_idx]
            si_idx += 1
            ssi = sp.sync_info
            sw = list(ssi.on_wait) if ssi is not None and ssi.on_wait else []
            su = list(ssi.on_update) if ssi is not None and ssi.on_update else []
            sp.sync_info = mybir.SyncInfo(on_wait=sw + [w], on_update=su)
        inst.sync_info = mybir.SyncInfo(
            on_wait=keep, on_update=list(si.on_update) if si.on_update else [])
    n_waits = {}
    for pos, inst in enumerate(glob):
        si = inst.sync_info
        n_waits[pos] = len(si.on_wait) if si is not None and si.on_wait else 0

    import bisect
    for pos, inst in enumerate(glob):
        tname = type(inst).__name__
        if tname in skip_types or "Branch" in tname:
            continue
        si = inst.sync_info
        max_waits = WAIT_CAPS.get(tname, DEFAULT_WAIT_CAP)
        if n_waits[pos] <= max_waits:
            continue
        # DVE/ACT are strict-FIFO in-order engines: a wait on a sem whose
        # increments all come from earlier non-DMA instructions of the same
        # engine is trivially satisfied -> drop it.
        eng = inst.engine
        waits = list(si.on_wait)
        if str(eng) in ("EngineType.DVE", "EngineType.Activation"):
            kept = []
            for w in waits:
                ups = sem_updaters.get(w.id, set())
                pp = producer_pos(w)
                if ups and all(e == eng and not d for (e, d) in ups) \
                        and 0 <= pp < pos:
                    continue  # redundant same-engine self-wait
                kept.append(w)
            waits = kept
            if len(waits) <= max_waits:
                inst.sync_info = mybir.SyncInfo(
                    on_wait=waits,
                    on_update=list(si.on_update) if si.on_update else [])
                n_waits[pos] = len(waits)
                continue
        # Greedy: hoist whichever waits find carriers until <= max_waits remain.
        waits = sorted(waits, key=producer_pos)  # easiest (earliest) first
        keep = []
        need_hoist = len(waits) - max_waits
        hoisted = 0
        for w in waits:
            if hoisted >= need_hoist:
                keep.append(w)
                continue
            pp = producer_pos(w)
            placed = False
            if pp >= 0:
                ep = eng_pos[inst.engine]
                i = bisect.bisect_left(ep, pos) - 1
                while i >= 0 and ep[i] > pp:
                    q = ep[i]
                    cand = glob[q]
                    cn = type(cand).__name__
                    if (n_waits[q] < WAIT_CAPS.get(cn, DEFAULT_WAIT_CAP)
                            and cn not in skip_types and "Branch" not in cn):
                        csi = cand.sync_info
                        cw = list(csi.on_wait) if csi is not None and csi.on_wait else []
                        cu = list(csi.on_update) if csi is not None and csi.on_update else []
                        cand.sync_info = mybir.SyncInfo(on_wait=cw + [w], on_update=cu)
                        n_waits[q] += 1
                        placed = True
                        break
                    i -= 1
            if placed:
                hoisted += 1
            else:
                keep.append(w)
        if len(keep) > max_waits:
            if lenient:
                keep = keep[-max_waits:]
            else:
                raise RuntimeError(
                    f"legalize_waits: {inst.name} ({tname}, pos {pos}) still "
                    f"has {len(keep)} waits: {[str(w) for w in keep]}")
        inst.sync_info = mybir.SyncInfo(
            on_wait=keep, on_update=list(si.on_update) if si.on_update else [])
        n_waits[pos] = len(keep)


# ------------------------- host-side preparation -------------------------

def _hilo(x32):
    hi = x32.astype(BF)
    lo = (x32 - hi.astype(np.float32)).astype(BF)
    return hi, lo


def _norm_hilo(x32):
    n = (x32.astype(np.float64) ** 2).sum(axis=1)
    nh = n.astype(np.float32).astype(BF)
    nl = (n - nh.astype(np.float64)).astype(np.float32).astype(BF)
    return nh, nl


def aug_query(pts):
    """[P,3] f32 -> [13,P] bf16: (ah, ah, al, |a|^2 hi/lo, 1, 1)."""
    ah, al = _hilo(pts)
    nh, nl = _norm_hilo(pts)
    one = np.ones(pts.shape[0], dtype=BF)
    rows = [ah[:, 0], ah[:, 1], ah[:, 2],
            ah[:, 0], ah[:, 1], ah[:, 2],
            al[:, 0], al[:, 1], al[:, 2],
            nh, nl, one, one]
    return np.ascontiguousarray(np.stack(rows, axis=0))


def aug_target(pts):
    """[P,3] f32 -> [13,P] bf16: (-2bh, -2bl, -2bh, 1, 1, |b|^2 hi/lo)."""
    bh, bl = _hilo(pts)
    m2h = (-2.0 * bh.astype(np.float32)).astype(BF)
    m2l = (-2.0 * bl.astype(np.float32)).astype(BF)
    nh, nl = _norm_hilo(pts)
    one = np.ones(pts.shape[0], dtype=BF)
    rows = [m2h[:, 0], m2h[:, 1], m2h[:, 2],
            m2l[:, 0], m2l[:, 1], m2l[:, 2],
            m2h[:, 0], m2h[:, 1], m2h[:, 2],
            one, one, nh, nl]
    return np.ascontiguousarray(np.stack(rows, axis=0))


def make_in_maps(pc1_0, pc1_1, pc1_3, pc2, cfg=None):
    cfg = cfg or FULL_CFG
    a10 = np.asarray(pc1_0, np.float32).reshape(-1, 3)
    a11 = np.asarray(pc1_1, np.float32).reshape(-1, 3)
    a13 = np.asarray(pc1_3, np.float32).reshape(-1)
    a2 = np.asarray(pc2, np.float32).reshape(-1, 3)
    a2f = np.asarray(pc2, np.float32).reshape(-1)

    Q2, Q10, Q11 = aug_query(a2), aug_query(a10), aug_query(a11)
    T10, T2, T11 = aug_target(a10), aug_target(a2), aug_target(a11)

    nqp, nq1, mf = cfg["nq_pc"], cfg["nq_11"], cfg["mse_free"]
    mse_n = 128 * mf
    in_maps = []
    for i in range(NCORES):
        in_maps.append({
            "q_pc2": np.ascontiguousarray(Q2[:, i * nqp:(i + 1) * nqp]),
            "q_pc10": np.ascontiguousarray(Q10[:, i * nqp:(i + 1) * nqp]),
            "q_pc11": np.ascontiguousarray(Q11[:, i * nq1:(i + 1) * nq1]),
            "t_pc10": T10, "t_pc2": T2, "t_pc11": T11,
            "mse_a": np.ascontiguousarray(
                a13[i * mse_n:(i + 1) * mse_n].reshape(128, mf)),
            "mse_b": np.ascontiguousarray(
                a2f[i * mse_n:(i + 1) * mse_n].reshape(128, mf)),
        })
    return in_maps


def combine(partials_list):
    """partials_list: per-core [1,8] arrays -> final scalar (np.float32)."""
    s = np.stack([np.asarray(p, np.float64).reshape(-1) for p in partials_list]).sum(0)
    cd = (s[0] + s[1]) / 16384.0
    seed = s[2] / 16384.0 + s[3] / 4096.0
    mse = s[4] / 49152.0
    return np.float32(mse + 0.5 * cd + seed)


_NC_CACHE = {}


def _get_nc():
    if "nc" not in _NC_CACHE:
        _NC_CACHE["nc"] = build_bass(FULL_CFG)
    return _NC_CACHE["nc"]


def make_runner(nc):
    """Persistent jitted SPMD executor for `nc` (the run_bass_via_pjrt flow,
    but with the jit + neff cached so repeat calls only pay dispatch+exec)."""
    import jax
    from jax.sharding import Mesh, PartitionSpec
    from jax.experimental.shard_map import shard_map
    from concourse import bass2jax
    from concourse.bass2jax import _bass_exec_p, partition_id_tensor

    bass2jax.install_neuronx_cc_hook()
    partition_name = nc.partition_id_tensor.name if nc.partition_id_tensor else None
    in_names, out_names, out_avals, zero_outs = [], [], [], []
    for alloc in nc.m.functions[0].allocations:
        if not isinstance(alloc, mybir.MemoryLocationSet):
            continue
        name = alloc.memorylocations[0].name
        if alloc.kind == "ExternalInput":
            if name != partition_name:
                in_names.append(name)
        elif alloc.kind == "ExternalOutput":
            out_names.append(name)
            shape = tuple(alloc.tensor_shape)
            dtype = mybir.dt.np(alloc.dtype)
            out_avals.append(jax.core.ShapedArray(shape, dtype))
            zero_outs.append(np.zeros(shape, dtype))
    n_params = len(in_names)
    n_outs = len(out_avals)
    all_names = in_names + out_names + ([partition_name] if partition_name else [])
    donate = tuple(range(n_params, n_params + n_outs))

    def _body(*args):
        operands = list(args)
        if partition_name is not None:
            operands.append(partition_id_tensor())
        return tuple(_bass_exec_p.bind(
            *operands, out_avals=tuple(out_avals), in_names=tuple(all_names),
            out_names=tuple(out_names), lowering_input_output_aliases=(),
            sim_require_finite=True, sim_require_nnan=True, nc=nc))

    devices = jax.devices()[:NCORES]
    mesh = Mesh(np.asarray(devices), ("core",))
    sharded = jax.jit(
        shard_map(_body, mesh=mesh,
                  in_specs=(PartitionSpec("core"),) * (n_params + n_outs),
                  out_specs=(PartitionSpec("core"),) * n_outs,
                  check_rep=False),
        donate_argnums=donate, keep_unused=True)

    def run(in_maps):
        per_core = [[np.asarray(m[n]) for n in in_names] for m in in_maps]
        concat_in = [np.concatenate([per_core[c][i] for c in range(NCORES)], axis=0)
                     for i in range(n_params)]
        concat_zeros = [np.zeros((NCORES * z.shape[0], *z.shape[1:]), z.dtype)
                        for z in zero_outs]
        outs = sharded(*concat_in, *concat_zeros)
        return [
            {name: np.asarray(outs[i]).reshape(NCORES, *out_avals[i].shape)[c]
             for i, name in enumerate(out_names)}
            for c in range(NCORES)
        ]

    return run


def _get_runner():
    if "runner" not in _NC_CACHE:
        _NC_CACHE["runner"] = make_runner(_get_nc())
    return _NC_CACHE["runner"]


def run_hw(in_maps, trace=False, **kw):
    nc = _get_nc()
    return run_bass_kernel_spmd(nc, in_maps, list(range(NCORES)), trace=trace, **kw)


def kernel(pc1_0, pc1_1, pc1_3, pc2):
    in_maps = make_in_maps(pc1_0, pc1_1, pc1_3, pc2)
    results = _get_runner()(in_maps)
    return combine([r["partials"] for r in results])


def build_null():
    """Minimal kernel over the same run path — dispatch/overhead baseline."""
    nc = bass.Bass()
    d_in = nc.declare_dram_parameter("x", [1, 8], F32, isOutput=False)
    d_out = nc.declare_dram_parameter("partials", [1, 8], F32, isOutput=True)
    with SplitDrainTileContext(nc) as tc:
        with tc.tile_pool(name="pin", bufs=1) as pin:
            t = pin.tile([1, 8], F32, tag="t")
            nc.sync.dma_start(t[:], d_in[:])
            nc.sync.dma_start(d_out[:], t[:])
    legalize_waits(nc)
    return nc


# revision 46
# speedup vs baseline: 493.3076x; 493.3076x over previous
"""Trainium2 Bass kernel for nn_CombinedLoss (chamfer x2 + MSE).

final = mse(pc1_3, pc2) + 0.5*chamfer(pc1_0, pc2) + chamfer(pc1_1, pc2)

Strategy (8 NeuronCores, SPMD):
  Four KNN "directions" (query set -> target set):
    D0: q=pc2    (16384) t=pc1_0  (16384)   [cd dist1]
    D1: q=pc1_0  (16384) t=pc2    (16384)   [cd dist2]
    D2: q=pc2    (16384) t=pc1_1  (4096)    [seed dist1]
    D3: q=pc1_1  (4096)  t=pc2    (16384)   [seed dist2]
  Queries of each direction are sharded across the 8 cores (row-block of the
  pairwise-distance matrix); each core computes sum(sqrt(min d2)) over its
  query rows entirely on-device, plus a partial MSE sum.  Host just sums the
  8x per-direction scalars and divides (the "all-reduce of means").

  d2 is produced directly by the tensor engine: points are augmented to
  K=13 bf16 hi/lo vectors such that aT@b = |a|^2 + |b|^2 - 2 a.b (exact to
  ~2^-16) and accumulated in fp32 PSUM.  Row-mins are maintained by DVE
  tensor_tensor_reduce ops with running-min chaining; optionally ScalarE
  casts PSUM tiles to fp16 in SBUF so the DVE reduce runs in 2x mode over
  pairs of tiles (4 entries/cycle/lane).
"""

import numpy as np
import ml_dtypes
from contextlib import ExitStack

import bass_rust
import concourse.bass as bass
import concourse.tile as tile
from concourse import mybir
from concourse.bass_utils import run_bass_kernel_spmd
from concourse.vector_clock import ScopedClock


class SplitDrainTileContext(tile.TileContext):
    """TileContext that emits spare bare drains before the tail drain.  The
    tail drain needs ~12 sync waits but HW instructions carry only one
    through this walrus backend; legalize_waits() redistributes the excess
    onto the recorded bare drains (safe: nothing depends on a bare drain)."""

    N_SPARE_DRAINS = 24

    def _drain_and_barrier(self, tick_clock, wait_clock):
        spares = []
        for _ in range(self.N_SPARE_DRAINS):
            d = self.nc.sync.drain()
            spares.append(d.ins.name if hasattr(d, "ins") else d.name)
        self.nc._spare_drain_names = set(spares)
        return super()._drain_and_barrier(tick_clock, wait_clock)

F32 = mybir.dt.float32
F16 = mybir.dt.float16
BF16 = mybir.dt.bfloat16
OP_MIN = mybir.AluOpType.min
OP_ADD = mybir.AluOpType.add
OP_SUB = mybir.AluOpType.subtract
OP_MUL = mybir.AluOpType.mult
AXIS_X = mybir.AxisListType.X
SQRT = mybir.ActivationFunctionType.Sqrt

NCORES = 8
K = 13          # augmented contraction dim
MMN = 512       # matmul free dim (one PSUM bank of fp32)
GRP = 2048      # targets per reduce group (4 banks)
QT = 128        # queries per tile (PE partition dim)
BIGF = 3.0e38

BF = ml_dtypes.bfloat16

# Full-problem config.  Per-core query counts; targets are full.
FULL_CFG = dict(
    nq_pc=2048,      # per-core slice of a 16384-point query set
    nq_11=512,       # per-core slice of the 4096-point query set
    nt_pc=16384,     # full target set size (pc1_0 / pc2)
    nt_11=4096,      # full target set size (pc1_1)
    mse_free=48,     # per-core MSE elements = 128 * mse_free
    # groups per query-tile routed through the ScalarE fp16-cast path
    # (DVE 4x reduce); the rest reduce directly from PSUM on DVE at 1x.
    cast_16k=5,      # for directions with 16384 targets (8 groups/tile)
    cast_4k=1,       # for directions with 4096 targets (2 groups/tile)
)


def build_bass(cfg, debug_taps=False):
    nc = bass.Bass()

    # Tile's tail sem-clear lowers to EVENT_SEMAPHORE_RANGE_CLEAR, which this
    # neuronxcc walrus rejects ("ISA wrong length").  NRT's per-execution
    # preamble already zeroes user semaphores (runtime sema_reset), so skip
    # emitting the clear instructions but keep the allocator bookkeeping.
    def _clear_and_free(sems, _nc=nc):
        if not sems:
            return
        sem_nums = [s.num if hasattr(s, "num") else s for s in sems]
        _nc._state.prepend_free_semaphores(sem_nums)
        for poison_set in _nc._tile_sem_poison_stack:
            poison_set.update(sem_nums)
    nc.clear_and_free_semaphores = _clear_and_free

    nq_pc, nq_11 = cfg["nq_pc"], cfg["nq_11"]
    nt_pc, nt_11 = cfg["nt_pc"], cfg["nt_11"]
    mse_free = cfg["mse_free"]

    d_q2 = nc.declare_dram_parameter("q_pc2", [K, nq_pc], BF16, isOutput=False)
    d_q10 = nc.declare_dram_parameter("q_pc10", [K, nq_pc], BF16, isOutput=False)
    d_q11 = nc.declare_dram_parameter("q_pc11", [K, nq_11], BF16, isOutput=False)
    d_t10 = nc.declare_dram_parameter("t_pc10", [K, nt_pc], BF16, isOutput=False)
    d_t2 = nc.declare_dram_parameter("t_pc2", [K, nt_pc], BF16, isOutput=False)
    d_t11 = nc.declare_dram_parameter("t_pc11", [K, nt_11], BF16, isOutput=False)
    d_ma = nc.declare_dram_parameter("mse_a", [128, mse_free], F32, isOutput=False)
    d_mb = nc.declare_dram_parameter("mse_b", [128, mse_free], F32, isOutput=False)
    d_out = nc.declare_dram_parameter("partials", [1, 8], F32, isOutput=True)
    d_dbg = {}

    # direction table: (query dram, n_queries, target dram, n_targets, cast_groups)
    cast16, cast4 = cfg["cast_16k"], cfg["cast_4k"]
    dirs = [
        (d_q2, nq_pc, d_t10, nt_pc, cast16),
        (d_q10, nq_pc, d_t2, nt_pc, cast16),
        (d_q2, nq_pc, d_t11, nt_11, cast4),
        (d_q11, nq_11, d_t2, nt_pc, cast16),
    ]
    n_tiles = [nq // QT for (_, nq, _, _, _) in dirs]
    n_grps = [nt // GRP for (_, _, _, nt, _) in dirs]
    ntot_tiles = sum(n_tiles)
    # one raw column per (query tile, target group) + one for MSE
    raw_base = []
    acc = 0
    for ntl, ng in zip(n_tiles, n_grps):
        raw_base.append(acc)
        acc += ntl * ng
    n_raw = acc + 1
    mse_col = n_raw - 1

    with SplitDrainTileContext(nc) as tc, ExitStack() as ctx:
        pin = ctx.enter_context(tc.tile_pool(name="pin", bufs=1))
        ppsum = ctx.enter_context(tc.tile_pool(name="ppsum", bufs=2, space="PSUM"))
        pcast = ctx.enter_context(tc.tile_pool(name="pcast", bufs=4))
        prm = ctx.enter_context(tc.tile_pool(name="prm", bufs=4))
        pout = ctx.enter_context(tc.tile_pool(name="pout", bufs=2))

        # --- resident inputs / constants ---
        sb_q = {}
        for name, dram, shape in (
            ("q2", d_q2, [K, nq_pc]),
            ("q10", d_q10, [K, nq_pc]),
            ("q11", d_q11, [K, nq_11]),
            ("t10", d_t10, [K, nt_pc]),
            ("t2", d_t2, [K, nt_pc]),
            ("t11", d_t11, [K, nt_11]),
        ):
            t = pin.tile(shape, BF16, tag=name)
            nc.gpsimd.dma_start(t[:], dram[:])
            sb_q[name] = t
        dram_to_sb = {id(d_q2): "q2", id(d_q10): "q10", id(d_q11): "q11",
                      id(d_t10): "t10", id(d_t2): "t2", id(d_t11): "t11"}

        ma = pin.tile([128, mse_free], F32, tag="ma")
        nc.gpsimd.dma_start(ma[:], d_ma[:])
        mb = pin.tile([128, mse_free], F32, tag="mb")
        nc.gpsimd.dma_start(mb[:], d_mb[:])

        ones = pin.tile([128, 1], F32, tag="ones")
        nc.vector.memset(ones[:], 1.0)
        res_raw = pin.tile([128, n_raw], F32, tag="resraw")
        mins = pin.tile([128, ntot_tiles], F32, tag="mins")
        sums = pin.tile([128, 8], F32, tag="sums")
        nc.vector.memset(sums[:], 0.0)

        # --- DMA-sem observers: each engine observes every input DMA once,
        # so no later compute instruction needs more than one sync wait. ---
        obs = pin.tile([1, 2], F32, tag="obs")
        for oi, t in enumerate((ma, mb)):
            nc.vector.tensor_copy(obs[:, oi:oi + 1], t[0:1, 0:1])
        for name in ("q2", "q10", "q11", "t10", "t2", "t11"):
            t = sb_q[name]
            wps = ppsum.tile([1, 1], F32, tag="grp")
            nc.tensor.matmul(wps[:], lhsT=t[:, 0:1], rhs=t[:, 0:1],
                             start=True, stop=True)

        # --- MSE partial: sum((a-b)^2) per partition -> res_raw[:, mse_col] ---
        diff = pin.tile([128, mse_free], F32, tag="diff")
        nc.vector.tensor_tensor(diff[:], ma[:], mb[:], OP_SUB)
        sq = pin.tile([128, mse_free], F32, tag="sq")
        nc.vector.tensor_tensor(sq[:], diff[:], diff[:], OP_MUL)
        nc.vector.tensor_reduce(res_raw[:, mse_col:mse_col + 1], sq[:],
                                mybir.AxisListType.X, OP_ADD)

        # --- chamfer directions ---
        for di, (d_qd, nq, d_td, nt, ncast) in enumerate(dirs):
            q_sb = sb_q[dram_to_sb[id(d_qd)]]
            t_sb = sb_q[dram_to_sb[id(d_td)]]
            ngrp = n_grps[di]
            ncast_d = min(ncast, ngrp)
            for ti in range(nq // QT):
                q_ap = q_sb[:, ti * QT:(ti + 1) * QT]
                for g in range(ngrp):
                    ps = ppsum.tile([128, GRP], F32, tag="grp")
                    for m in range(GRP // MMN):
                        off = g * GRP + m * MMN
                        nc.tensor.matmul(
                            ps[:, m * MMN:(m + 1) * MMN],
                            lhsT=q_ap, rhs=t_sb[:, off:off + MMN],
                            start=True, stop=True,
                        )
                    gc = raw_base[di] + ti * ngrp + g
                    acc_ap = res_raw[:, gc:gc + 1]
                    if g < ncast_d:
                        ct = pcast.tile([128, GRP], F16, tag="ct")
                        # 1-element ACT toucher: absorbs the WAR-on-slot wait
                        # (vs the DVE reader of the slot's previous tenant) so
                        # the real cast carries only its PE wait (HW instrs
                        # hold a single sync-wait slot).
                        nc.scalar.mul(ct[0:1, 0:1], ct[0:1, 0:1], 0.0)
                        nc.scalar.copy(ct[:], ps[:])
                        to = pout.tile([128, GRP], F16, tag="ttr_out")
                        nc.vector.tensor_scalar(
                            to[:], ct[:], BIGF, None, OP_MIN, OP_MIN,
                            accum_out=acc_ap)
                    else:
                        to = pout.tile([128, GRP], F32, tag="ttr_out")
                        nc.vector.tensor_scalar(
                            to[:], ps[:], BIGF, None, OP_MIN, OP_MIN,
                            accum_out=acc_ap)

        def tap(nm, tl, shape, dt_):
            if debug_taps:
                d = nc.declare_dram_parameter(nm, shape, dt_, isOutput=True)
                nc.sync.dma_start(d[:], tl[:])

        # --- finals: clamp, per-tile min over groups, sqrt, sums ---
        ngc = n_raw - 1
        nc.vector.tensor_scalar_max(res_raw[:, 0:ngc], res_raw[:, 0:ngc], 0.0)
        tap("dbg_raw", res_raw, [128, n_raw], F32)
        c0 = 0
        for di, (ntl, ng) in enumerate(zip(n_tiles, n_grps)):
            base = raw_base[di]
            src = res_raw[:, base:base + ntl * ng]
            src3 = src.rearrange("p (t g) -> p t g", g=ng)
            nc.vector.tensor_reduce(mins[:, c0:c0 + ntl], src3,
                                    mybir.AxisListType.X, OP_MIN)
            c0 += ntl
        nc.scalar.activation(mins[:, 0:ntot_tiles], mins[:, 0:ntot_tiles], SQRT)
        tap("dbg_mins", mins, [128, ntot_tiles], F32)
        c0 = 0
        for d, ntl in enumerate(n_tiles):
            nc.vector.reduce_sum(sums[:, d:d + 1], mins[:, c0:c0 + ntl], axis=AXIS_X)
            c0 += ntl
        nc.vector.tensor_copy(sums[:, 4:5], res_raw[:, mse_col:mse_col + 1])

        ps_fin = ppsum.tile([1, 8], F32, tag="grp")
        nc.tensor.matmul(ps_fin[:], lhsT=ones[:], rhs=sums[:], start=True, stop=True)
        out_sb = pin.tile([1, 8], F32, tag="outsb")
        nc.vector.tensor_copy(out_sb[:], ps_fin[:])
        nc.sync.dma_start(d_out[:], out_sb[:])

    legalize_waits(nc, lenient=debug_taps)
    return nc


WAIT_CAPS = {}
DEFAULT_WAIT_CAP = 1


def legalize_waits(nc, skip_types=("InstDrain",), lenient=False):
    """Cap per-instruction sync waits for the neuronxcc walrus backend.

    HW instruction structs carry a single (wait, update) EVENTS slot; walrus
    rejects instructions (at least matmuls) with more than one wait.  Excess
    waits are hoisted onto an earlier instruction of the same engine that has
    a free wait slot.  Safety: a hoisted wait may only move to a position
    after the instruction whose sem update satisfies it (positions taken in
    global block order = Tile's scheduled order, a valid topological order),
    so the schedule itself remains feasible and no deadlock is introduced.
    """
    f = nc.m.functions[0]
    glob = []
    for blk in f.blocks:
        for inst in blk.instructions:
            glob.append(inst)

    # cumulative sem updates in scheduled order
    from collections import defaultdict
    cum = defaultdict(int)
    hist = defaultdict(list)  # sem id -> [(pos, cum_after)]
    sem_updaters = defaultdict(set)  # sem id -> {(engine, is_dma)}
    for pos, inst in enumerate(glob):
        si = inst.sync_info
        if si is not None and si.on_update:
            is_dma = type(inst).__name__ == "InstDMACopy"
            for u in si.on_update:
                cum[u.id] += u.update_value if u.update_value is not None else 1
                hist[u.id].append((pos, cum[u.id]))
                sem_updaters[u.id].add((inst.engine, is_dma))

    def producer_pos(w):
        for pos, c in hist[w.id]:
            if c >= w.wait_value:
                return pos
        return -1  # satisfied externally / never: be conservative below

    eng_pos = defaultdict(list)  # engine -> [global positions]
    for pos, inst in enumerate(glob):
        eng_pos[inst.engine].append(pos)

    n_waits = {}
    for pos, inst in enumerate(glob):
        si = inst.sync_info
        n_waits[pos] = len(si.on_wait) if si is not None and si.on_wait else 0

    # The tail drain aggregates the whole global clock (~12 waits).  Move its
    # excess waits onto the spare bare drains emitted just before it; nothing
    # depends on a bare drain, so this cannot deadlock.
    spare_names = getattr(nc, "_spare_drain_names", set())
    spares = [i for i in glob if i.name in spare_names]
    si_idx = 0
    for pos, inst in enumerate(glob):
        if type(inst).__name__ != "InstDrain" or inst.name in spare_names:
            continue
        si = inst.sync_info
        if si is None or not si.on_wait or len(si.on_wait) <= 1:
            continue
        waits = list(si.on_wait)
        keep = waits[:1]
        for w in waits[1:]:
            if si_idx >= len(spares):
                keep.append(w)
                continue
            sp = spares[si_idx]
            si_idx += 1
            ssi = sp.sync_info
            sw = list(ssi.on_wait) if ssi is not None and ssi.on_wait else []
            su = list(ssi.on_update) if ssi is not None and ssi.on_update else []
            sp.sync_info = mybir.SyncInfo(on_wait=sw + [w], on_update=su)
        inst.sync_info = mybir.SyncInfo(
            on_wait=keep, on_update=list(si.on_update) if si.on_update else [])
    n_waits = {}
    for pos, inst in enumerate(glob):
        si = inst.sync_info
        n_waits[pos] = len(si.on_wait) if si is not None and si.on_wait else 0

    import bisect
    for pos, inst in enumerate(glob):
        tname = type(inst).__name__
        if tname in skip_types or "Branch" in tname:
            continue
        si = inst.sync_info
        max_waits = WAIT_CAPS.get(tname, DEFAULT_WAIT_CAP)
        if n_waits[pos] <= max_waits:
            continue
        # DVE/ACT are strict-FIFO in-order engines: a wait on a sem whose
        # increments all come from earlier non-DMA instructions of the same
        # engine is trivially satisfied -> drop it.
        eng = inst.engine
        waits = list(si.on_wait)
        if str(eng) in ("EngineType.DVE", "EngineType.Activation"):
            kept = []
            for w in waits:
                ups = sem_updaters.get(w.id, set())
                pp = producer_pos(w)
                if ups and all(e == eng and not d for (e, d) in ups) \
                        and 0 <= pp < pos:
                    continue  # redundant same-engine self-wait
                kept.append(w)
            waits = kept
            if len(waits) <= max_waits:
                inst.sync_info = mybir.SyncInfo(
                    on_wait=waits,
                    on_update=list(si.on_update) if si.on_update else [])
                n_waits[pos] = len(waits)
                continue
        # Greedy: hoist whichever waits find carriers until <= max_waits remain.
        waits = sorted(waits, key=producer_pos)  # easiest (earliest) first
        keep = []
        need_hoist = len(waits) - max_waits
        hoisted = 0
        for w in waits:
            if hoisted >= need_hoist:
                keep.append(w)
                continue
            pp = producer_pos(w)
            placed = False
            if pp >= 0:
                ep = eng_pos[inst.engine]
                i = bisect.bisect_left(ep, pos) - 1
                while i >= 0 and ep[i] > pp:
                    q = ep[i]
                    cand = glob[q]
                    cn = type(cand).__name__
                    if (n_waits[q] < WAIT_CAPS.get(cn, DEFAULT_WAIT_CAP)
                            and cn not in skip_types and "Branch" not in cn):
                        csi = cand.sync_info
                        cw = list(csi.on_wait) if csi is not None and csi.on_wait else []
                        cu = list(csi.on_update) if csi is not None and csi.on_update else []
                        cand.sync_info = mybir.SyncInfo(on_wait=cw + [w], on_update=cu)
                        n_waits[q] += 1
                        placed = True
                        break
                    i -= 1
            if placed:
                hoisted += 1
            else:
                keep.append(w)
        if len(keep) > max_waits:
            if lenient:
                keep = keep[-max_waits:]
            else:
                raise RuntimeError(
                    f"legalize_waits: {inst.name} ({tname}, pos {pos}) still "
                    f"has {len(keep)} waits: {[str(w) for w in keep]}")
        inst.sync_info = mybir.SyncInfo(
            on_wait=keep, on_update=list(si.on_update) if si.on_update else [])
        n_waits[pos] = len(keep)


# ------------------------- host-side preparation -------------------------

def _hilo(x32):
    hi = x32.astype(BF)
    lo = (x32 - hi.astype(np.float32)).astype(BF)
    return hi, lo


def _norm_hilo(x32):
    n = (x32.astype(np.float64) ** 2).sum(axis=1)
    nh = n.astype(np.float32).astype(BF)
    nl = (n - nh.astype(np.float64)).astype(np.float32).astype(BF)
    return nh, nl


def aug_query(pts):
    """[P,3] f32 -> [13,P] bf16: (ah, ah, al, |a|^2 hi/lo, 1, 1)."""
    ah, al = _hilo(pts)
    nh, nl = _norm_hilo(pts)
    one = np.ones(pts.shape[0], dtype=BF)
    rows = [ah[:, 0], ah[:, 1], ah[:, 2],
            ah[:, 0], ah[:, 1], ah[:, 2],
            al[:, 0], al[:, 1], al[:, 2],
            nh, nl, one, one]
    return np.ascontiguousarray(np.stack(rows, axis=0))


def aug_target(pts):
    """[P,3] f32 -> [13,P] bf16: (-2bh, -2bl, -2bh, 1, 1, |b|^2 hi/lo)."""
    bh, bl = _hilo(pts)
    m2h = (-2.0 * bh.astype(np.float32)).astype(BF)
    m2l = (-2.0 * bl.astype(np.float32)).astype(BF)
    nh, nl = _norm_hilo(pts)
    one = np.ones(pts.shape[0], dtype=BF)
    rows = [m2h[:, 0], m2h[:, 1], m2h[:, 2],
            m2l[:, 0], m2l[:, 1], m2l[:, 2],
            m2h[:, 0], m2h[:, 1], m2h[:, 2],
            one, one, nh, nl]
    return np.ascontiguousarray(np.stack(rows, axis=0))


def make_in_maps(pc1_0, pc1_1, pc1_3, pc2, cfg=None):
    cfg = cfg or FULL_CFG
    a10 = np.asarray(pc1_0, np.float32).reshape(-1, 3)
    a11 = np.asarray(pc1_1, np.float32).reshape(-1, 3)
    a13 = np.asarray(pc1_3, np.float32).reshape(-1)
    a2 = np.asarray(pc2, np.float32).reshape(-1, 3)
    a2f = np.asarray(pc2, np.float32).reshape(-1)

    Q2, Q10, Q11 = aug_query(a2), aug_query(a10), aug_query(a11)
    T10, T2, T11 = aug_target(a10), aug_target(a2), aug_target(a11)

    nqp, nq1, mf = cfg["nq_pc"], cfg["nq_11"], cfg["mse_free"]
    mse_n = 128 * mf
    in_maps = []
    for i in range(NCORES):
        in_maps.append({
            "q_pc2": np.ascontiguousarray(Q2[:, i * nqp:(i + 1) * nqp]),
            "q_pc10": np.ascontiguousarray(Q10[:, i * nqp:(i + 1) * nqp]),
            "q_pc11": np.ascontiguousarray(Q11[:, i * nq1:(i + 1) * nq1]),
            "t_pc10": T10, "t_pc2": T2, "t_pc11": T11,
            "mse_a": np.ascontiguousarray(
                a13[i * mse_n:(i + 1) * mse_n].reshape(128, mf)),
            "mse_b": np.ascontiguousarray(
                a2f[i * mse_n:(i + 1) * mse_n].reshape(128, mf)),
        })
    return in_maps


def combine(partials_list):
    """partials_list: per-core [1,8] arrays -> final scalar (np.float32)."""
    s = np.stack([np.asarray(p, np.float64).reshape(-1) for p in partials_list]).sum(0)
    cd = (s[0] + s[1]) / 16384.0
    seed = s[2] / 16384.0 + s[3] / 4096.0
    mse = s[4] / 49152.0
    return np.float32(mse + 0.5 * cd + seed)


_NC_CACHE = {}


def _get_nc():
    if "nc" not in _NC_CACHE:
        _NC_CACHE["nc"] = build_bass(FULL_CFG)
    return _NC_CACHE["nc"]


def make_runner(nc):
    """Persistent jitted SPMD executor for `nc` (the run_bass_via_pjrt flow,
    but with the jit + neff cached so repeat calls only pay dispatch+exec)."""
    import jax
    from jax.sharding import Mesh, PartitionSpec
    from jax.experimental.shard_map import shard_map
    from concourse import bass2jax
    from concourse.bass2jax import _bass_exec_p, partition_id_tensor

    bass2jax.install_neuronx_cc_hook()
    partition_name = nc.partition_id_tensor.name if nc.partition_id_tensor else None
    in_names, out_names, out_avals, zero_outs = [], [], [], []
    for alloc in nc.m.functions[0].allocations:
        if not isinstance(alloc, mybir.MemoryLocationSet):
            continue
        name = alloc.memorylocations[0].name
        if alloc.kind == "ExternalInput":
            if name != partition_name:
                in_names.append(name)
        elif alloc.kind == "ExternalOutput":
            out_names.append(name)
            shape = tuple(alloc.tensor_shape)
            dtype = mybir.dt.np(alloc.dtype)
            out_avals.append(jax.core.ShapedArray(shape, dtype))
            zero_outs.append(np.zeros(shape, dtype))
    n_params = len(in_names)
    n_outs = len(out_avals)
    all_names = in_names + out_names + ([partition_name] if partition_name else [])
    donate = tuple(range(n_params, n_params + n_outs))

    def _body(*args):
        operands = list(args)
        if partition_name is not None:
            operands.append(partition_id_tensor())
        return tuple(_bass_exec_p.bind(
            *operands, out_avals=tuple(out_avals), in_names=tuple(all_names),
            out_names=tuple(out_names), lowering_input_output_aliases=(),
            sim_require_finite=True, sim_require_nnan=True, nc=nc))

    devices = jax.devices()[:NCORES]
    mesh = Mesh(np.asarray(devices), ("core",))
    sharded = jax.jit(
        shard_map(_body, mesh=mesh,
                  in_specs=(PartitionSpec("core"),) * (n_params + n_outs),
                  out_specs=(PartitionSpec("core"),) * n_outs,
                  check_rep=False),
        donate_argnums=donate, keep_unused=True)

    def run(in_maps):
        per_core = [[np.asarray(m[n]) for n in in_names] for m in in_maps]
        concat_in = [np.concatenate([per_core[c][i] for c in range(NCORES)], axis=0)
                     for i in range(n_params)]
        concat_zeros = [np.zeros((NCORES * z.shape[0], *z.shape[1:]), z.dtype)
                        for z in zero_outs]
        outs = sharded(*concat_in, *concat_zeros)
        return [
            {name: np.asarray(outs[i]).reshape(NCORES, *out_avals[i].shape)[c]
             for i, name in enumerate(out_names)}
            for c in range(NCORES)
        ]

    return run


def _get_runner():
    if "runner" not in _NC_CACHE:
        _NC_CACHE["runner"] = make_runner(_get_nc())
    return _NC_CACHE["runner"]


def run_hw(in_maps, trace=False, **kw):
    nc = _get_nc()
    return run_bass_kernel_spmd(nc, in_maps, list(range(NCORES)), trace=trace, **kw)


def kernel(pc1_0, pc1_1, pc1_3, pc2):
    in_maps = make_in_maps(pc1_0, pc1_1, pc1_3, pc2)
    results = _get_runner()(in_maps)
    return combine([r["partials"] for r in results])


def make_chain_runner(nc, nchain):
    """Like make_runner, but executes the NEFF `nchain` times back-to-back
    inside one jit (each round's outputs feed the next round's donated output
    buffers, forcing sequential execution).  Timing two chain lengths and
    taking the slope isolates pure HW exec time from dispatch/transfer."""
    import jax
    from jax.sharding import Mesh, PartitionSpec
    from jax.experimental.shard_map import shard_map
    from concourse import bass2jax
    from concourse.bass2jax import _bass_exec_p, partition_id_tensor

    bass2jax.install_neuronx_cc_hook()
    partition_name = nc.partition_id_tensor.name if nc.partition_id_tensor else None
    in_names, out_names, out_avals, zero_outs = [], [], [], []
    for alloc in nc.m.functions[0].allocations:
        if not isinstance(alloc, mybir.MemoryLocationSet):
            continue
        name = alloc.memorylocations[0].name
        if alloc.kind == "ExternalInput":
            if name != partition_name:
                in_names.append(name)
        elif alloc.kind == "ExternalOutput":
            out_names.append(name)
            shape = tuple(alloc.tensor_shape)
            dtype = mybir.dt.np(alloc.dtype)
            out_avals.append(jax.core.ShapedArray(shape, dtype))
            zero_outs.append(np.zeros(shape, dtype))
    n_params = len(in_names)
    n_outs = len(out_avals)
    all_names = in_names + out_names + ([partition_name] if partition_name else [])
    donate = tuple(range(n_params, n_params + n_outs))

    def _body(*args):
        ins = list(args[:n_params])
        outs = list(args[n_params:n_params + n_outs])
        for _ in range(nchain):
            operands = ins + outs
            if partition_name is not None:
                operands.append(partition_id_tensor())
            outs = list(_bass_exec_p.bind(
                *operands, out_avals=tuple(out_avals), in_names=tuple(all_names),
                out_names=tuple(out_names), lowering_input_output_aliases=(),
                sim_require_finite=True, sim_require_nnan=True, nc=nc))
        return tuple(outs)

    devices = jax.devices()[:NCORES]
    mesh = Mesh(np.asarray(devices), ("core",))
    sharded = jax.jit(
        shard_map(_body, mesh=mesh,
                  in_specs=(PartitionSpec("core"),) * (n_params + n_outs),
                  out_specs=(PartitionSpec("core"),) * n_outs,
                  check_rep=False),
        donate_argnums=donate, keep_unused=True)

    def run(in_maps):
        per_core = [[np.asarray(m[n]) for n in in_names] for m in in_maps]
        concat_in = [np.concatenate([per_core[c][i] for c in range(NCORES)], axis=0)
                     for i in range(n_params)]
        concat_zeros = [np.zeros((NCORES * z.shape[0], *z.shape[1:]), z.dtype)
                        for z in zero_outs]
        outs = sharded(*concat_in, *concat_zeros)
        return [
            {name: np.asarray(outs[i]).reshape(NCORES, *out_avals[i].shape)[c]
             for i, name in enumerate(out_names)}
            for c in range(NCORES)
        ]

    return run


def build_null():
    """Minimal kernel over the same run path — dispatch/overhead baseline."""
    nc = bass.Bass()
    d_in = nc.declare_dram_parameter("x", [1, 8], F32, isOutput=False)
    d_out = nc.declare_dram_parameter("partials", [1, 8], F32, isOutput=True)
    with SplitDrainTileContext(nc) as tc:
        with tc.tile_pool(name="pin", bufs=1) as pin:
            t = pin.tile([1, 8], F32, tag="t")
            nc.sync.dma_start(t[:], d_in[:])
            nc.sync.dma_start(d_out[:], t[:])
    legalize_waits(nc)
    return nc
